# revision 14
# baseline (speedup 1.0000x reference)
"""CKANKANNet Trainium2 kernel (per-core SPMD program, B=8 samples/core).

Basis algorithm (d3-direct): with v = 2.5x+5.5 mapped to integer knots and
V = min(v, 11) via RY = relu(5.5-2.5x), the 3rd finite difference of relu
cubes saturates: d3_m(t) = t^3 - 3 relu(t-1)^3 + 3 relu(t-2)^3 for
t = clamp(V-m, 0, 3), constant 6 beyond. Each d3 slab comes from one scalar
relu (t_m), one DVE tensor_scalar clamp (u_m = max(-t_m, -3)), and two fused
custom DVE ops (D3A: 3*min(u+1,0)^3 - u^3; D3B: A - 3*min(u+2,0)^3).
basis*6 = d4_j = d3_j - d3_{j+1}; the /6 is folded into the weights.

Convs: fp16 matmuls, channels on K partitions, 3x3 taps as accumulating
matmuls with edge-trimmed N ranges. L1 bakes ky taps into K via 3 shifted
channel-block copies (K=96). Image halves are {0,1,4,5}/{2,3,6,7} at every
layer so each layer's half-0 compute overlaps the previous layer's half-1.
Linear uses j-pair packed K=128 stationaries.
"""
import sys
sys.path.insert(0, '/opt/trn_rl_repo')
from contextlib import ExitStack

import numpy as np
MM_NP = np.float16

import concourse.bass as bass
import concourse.tile as tile
from concourse import bacc, mybir
from concourse import dve_ops
from concourse.dve_spec import (Spec, Src0, Src1, sq, lower, minn, _has_src1,
                                C0, C1, C2, Zero)
from concourse.dve_uop import DveOpSpec

F32 = mybir.dt.float32
F16 = mybir.dt.float16
MMDT = mybir.dt.float16
AF = mybir.ActivationFunctionType
OP = mybir.AluOpType

B = 8
NB = 8
ND3 = 9
O_OUT = 100
IMGS = [[0, 1, 4, 5], [2, 3, 6, 7]]  # half -> image ids (all layers)


# ------------------------------------------------------------- custom DVE ops
def _register_dve_op(name, spec, subdim=False):
    if name in dve_ops._SUB_OPCODE_FOR_NAME:
        return next(op for op in dve_ops.OPS if op.name == name)
    row = max(dve_ops._SUB_OPCODE_FOR_NAME.values()) + 1
    assert row < 0x20
    ver = 'v3'
    tmp = DveOpSpec(name=name, opcode=row, uops=lower(spec, ver=ver),
                    rd1_en=_has_src1(spec))
    op = dve_ops.DveOp(name, spec, subdim, uops_sha={ver: tmp.sha(ver)})
    dve_ops.OPS.append(op)
    dve_ops.CUSTOM_DVE_SPECS[name] = spec
    dve_ops._SUB_OPCODE_FOR_NAME[name] = row
    return op


def _d3a_ref(in0, in1, s0, s1, imm2):
    u = in0.astype(np.float32)
    m1 = np.minimum(u + s0, 0)
    return (imm2 * (m1 * m1 * m1) - u * u * u).astype(np.float32)


def _d3b_ref(in0, in1, s0, s1, imm2):
    u = in0.astype(np.float32)
    m2 = np.minimum(u + s0, 0)
    return (in1.astype(np.float32) - s1 * (m2 * m2 * m2)).astype(np.float32)


_m1 = minn(Src0 + C0, Zero)
D3A = _register_dve_op('D3A', Spec(body=sq(_m1) * _m1 * C2 - sq(Src0) * Src0,
                                   reference=_d3a_ref))
_m2 = minn(Src0 + C0, Zero)
D3B = _register_dve_op('D3B', Spec(body=Src1 - sq(_m2) * _m2 * C1,
                                   reference=_d3b_ref))


# ------------------------------------------------------------- weight folding
def silu_np(x):
    return x / (1.0 + np.exp(-x))


def fold_weights(wb1, ws1, wb2, ws2, wb3, ws3, lb, lc):
    out = {}
    W1 = np.zeros((96, 3 * 64), np.float32)
    for kyi in range(3):
        for kxi in range(3):
            W1[kyi * 32 + 0:kyi * 32 + 3, kxi * 64:(kxi + 1) * 64] = wb1[:, :, kyi, kxi].T
            blk = np.transpose(ws1[:, :, kyi, kxi].reshape(64, 3, NB), (2, 1, 0)) / 6.0
            W1[kyi * 32 + 3:kyi * 32 + 27, kxi * 64:(kxi + 1) * 64] = blk.reshape(24, 64)
    out['w1'] = W1.astype(MM_NP)

    W2 = np.zeros((576, 9 * 128), np.float32)
    for kyi in range(3):
        for kxi in range(3):
            t = kyi * 3 + kxi
            blk = np.transpose(ws2[:, :, kyi, kxi].reshape(128, 64, NB), (2, 1, 0)) / 6.0
            W2[0:512, t * 128:(t + 1) * 128] = blk.reshape(512, 128)
            W2[512:576, t * 128:(t + 1) * 128] = wb2[:, :, kyi, kxi].T
    out['w2'] = W2.astype(MM_NP)

    W3 = np.zeros((1152, 9 * 64), np.float32)
    for kyi in range(3):
        for kxi in range(3):
            t = kyi * 3 + kxi
            blk = np.transpose(ws3[:, :, kyi, kxi].reshape(64, 128, NB), (2, 1, 0)) / 6.0
            W3[0:1024, t * 64:(t + 1) * 64] = blk.reshape(1024, 64)
            W3[1024:1152, t * 64:(t + 1) * 64] = wb3[:, :, kyi, kxi].T
    out['w3'] = W3.astype(MM_NP)

    # linear: j-pair packed stationaries (K=128) + base (K=64)
    lc_r = lc.reshape(O_OUT, 64, 64, NB)
    lb_r = lb.reshape(O_OUT, 64, 64)
    WLP = np.zeros((128, 4, 16, 4, O_OUT), np.float32)
    WLB = np.zeros((64, 4, 16, O_OUT), np.float32)
    for p in range(4):
        for yi in range(16):
            yx = p * 16 + yi
            for pr in range(4):
                WLP[0:64, p, yi, pr, :] = lc_r[:, :, yx, 2 * pr].T / 6.0
                WLP[64:128, p, yi, pr, :] = lc_r[:, :, yx, 2 * pr + 1].T / 6.0
            WLB[:, p, yi, :] = lb_r[:, :, yx].T
    out['wlp'] = WLP.reshape(128, 4 * 16 * 4 * O_OUT).astype(MM_NP)
    out['wlb'] = WLB.reshape(64, 4 * 16 * O_OUT).astype(MM_NP)
    return out


# ------------------------------------------------------------- basis emission
def emit_d3(nc, tpool, d3pool, src_ap, P, E, bias_tiles, name=""):
    """src_ap: [P, E] activations. Returns D3 tile [P, 9E] fp16."""
    RY = tpool.tile([P, E], F32, tag="ry", name=f"RY{name}")
    nc.scalar.activation(RY[:], src_ap, AF.Relu, bias=bias_tiles['b55'][0:P, :],
                         scale=-2.5)
    D3 = d3pool.tile([P, ND3 * E], F16, tag="d3", name=f"D3{name}")
    for m in range(ND3):
        T = tpool.tile([P, E], F32, tag="t", name=f"T{name}_{m}")
        nc.scalar.activation(T[:], RY[:], AF.Relu, bias=bias_tiles[m][0:P, :],
                             scale=-1.0)
        U = tpool.tile([P, E], F32, tag="u", name=f"U{name}_{m}")
        nc.vector.tensor_scalar(U[:], T[:], -1.0, -3.0, op0=OP.mult, op1=OP.max)
        A = tpool.tile([P, E], F32, tag="a", name=f"A{name}_{m}")
        nc.vector._custom_dve(D3A, out=A[:], in0=U[:], s0=1.0, imm2=3.0)
        nc.vector._custom_dve(D3B, out=D3[:, m * E:(m + 1) * E], in0=U[:],
                              in1=A[:], s0=2.0, s1=3.0)
    return D3


def maxpool_from_psum(nc, psum_ap, n_bh, W_half, out_ap):
    pv = psum_ap.rearrange("p (hp r2 wp c2) -> p hp wp r2 c2",
                           hp=n_bh, r2=2, wp=W_half, c2=2)
    nc.vector.tensor_reduce(out_ap.rearrange("p (hp wp) -> p hp wp", wp=W_half),
                            pv, mybir.AxisListType.XY, OP.max, opt_input=False)


# ----------------------------------------------------------------- the kernel
def build_nc(dbg=()):
    nc = bacc.Bacc("TRN2", target_bir_lowering=False, debug=False, num_devices=8)
    x_ext = nc.declare_dram_parameter("x", [B, 3, 64, 64], F32, isOutput=False)
    w1_ext = nc.declare_dram_parameter("w1", [96, 192], MMDT, isOutput=False)
    w2_ext = nc.declare_dram_parameter("w2", [576, 1152], MMDT, isOutput=False)
    w3_ext = nc.declare_dram_parameter("w3", [1152, 576], MMDT, isOutput=False)
    wlp_ext = nc.declare_dram_parameter("wlp", [128, 25600], MMDT, isOutput=False)
    wlb_ext = nc.declare_dram_parameter("wlb", [64, 6400], MMDT, isOutput=False)
    out_ext = nc.declare_dram_parameter("out", [B, O_OUT], F32, isOutput=True)

    dbg_exts = {}

    def dbg_tap(name, shape, dt=F32):
        if name in dbg:
            dbg_exts[name] = nc.declare_dram_parameter(f"dbg_{name}", shape, dt, isOutput=True)
            return dbg_exts[name]
        return None

    with tile.TileContext(nc) as tc, ExitStack() as ctx:
        persist = ctx.enter_context(tc.tile_pool(name="persist", bufs=1))
        wpool = ctx.enter_context(tc.tile_pool(name="wpool", bufs=1))
        tpool = ctx.enter_context(tc.tile_pool(name="tpool", bufs=3))
        d3pool = ctx.enter_context(tc.tile_pool(name="d3pool", bufs=2))
        d4pool = ctx.enter_context(tc.tile_pool(name="d4pool", bufs=2))

        bias_tiles = {}
        for m in range(ND3):
            bt_m = wpool.tile([128, 1], F32, tag=f"bias_{m}", name=f"bias{m}")
            nc.gpsimd.memset(bt_m[:], float(11 - m))
            bias_tiles[m] = bt_m
        bt_55 = wpool.tile([128, 1], F32, tag="bias_55", name="bias55")
        nc.gpsimd.memset(bt_55[:], 5.5)
        bias_tiles['b55'] = bt_55
        w1sb = wpool.tile([96, 192], MMDT)
        nc.sync.dma_start(w1sb[:], w1_ext.ap())
        zt = wpool.tile([128, 2064], MMDT)
        nc.gpsimd.memset(zt[:], 0.0)

        h1 = persist.tile([64, 8192], F16)
        h2 = persist.tile([128, 2048], F32)
        h3 = persist.tile([64, 512], F32)

        # ================= L1 =================
        with tc.tile_pool(name="l1pool", bufs=1) as l1p:
            X1 = l1p.tile([128, 768], F32)
            for c in range(3):
                nc.sync.dma_start(
                    X1[:, c * 256:(c + 1) * 256],
                    x_ext.ap()[:, c, :, :].rearrange("b (g hh) w -> b g (hh w)", g=16))
            sl1 = l1p.tile([128, 768], MMDT)
            nc.scalar.activation(sl1[:], X1[:], AF.Silu)

            D4_1 = l1p.tile([128, NB * 768], MMDT)
            d41_v = D4_1[:].rearrange("p (j e) -> p j e", j=NB)
            for ck in range(2):
                E = 384
                D3_1 = emit_d3(nc, tpool, d3pool, X1[:, ck * E:(ck + 1) * E],
                               128, E, bias_tiles, name="l1")
                nc.vector.tensor_tensor(
                    d41_v[:, :, ck * E:(ck + 1) * E],
                    D3_1[:, 0:NB * E].rearrange("p (j e) -> p j e", j=NB),
                    D3_1[:, E:ND3 * E].rearrange("p (j e) -> p j e", j=NB),
                    op=OP.subtract)

            if (t := dbg_tap('d41', [128, NB * 768], MMDT)) is not None:
                nc.sync.dma_start(t.ap(), D4_1[:])

            # dump channels to DRAM (ch-major), then read back partition-
            # parallel into the three shifted ky blocks (32-aligned, K=96).
            l1ch = nc.dram_tensor("l1ch", [27, B * 4096], MMDT)
            for c in range(3):
                nc.sync.dma_start(
                    l1ch.ap()[c, :].rearrange("(bg e) -> bg e", e=256),
                    sl1[:, c * 256:(c + 1) * 256])
            for j in range(NB):
                nc.sync.dma_start(
                    l1ch.ap()[3 + j * 3:3 + j * 3 + 3, :]
                        .rearrange("c (bg e) -> bg c e", e=256),
                    D4_1[:, j * 768:(j + 1) * 768]
                        .rearrange("p (c e) -> p c e", e=256))

            BH = 4  # images per half
            for bh in range(2):
                imgs = IMGS[bh]
                Bun1 = l1p.tile([96, 64 + BH * 4096 + 64], MMDT,
                                tag=f"bun1_{bh}", name=f"Bun1_{bh}")
                for kyi in range(3):
                    base = 64 + (1 - kyi) * 64
                    # two image-pair spans: (2bh,2bh+1) and (4+2bh,4+2bh+1)
                    for sp in range(2):
                        b0 = imgs[2 * sp]
                        nc.sync.dma_start(
                            Bun1[kyi * 32:kyi * 32 + 27,
                                 base + sp * 2 * 4096:base + (sp + 1) * 2 * 4096],
                            l1ch.ap()[:, b0 * 4096:(b0 + 2) * 4096])
                    for k in range(8):
                        nc.sync.dma_start(
                            Bun1[kyi * 32 + 27:kyi * 32 + 32,
                                 k * 2064:(k + 1) * 2064],
                            zt[0:5, :])
                # boundary rows: ky=0 block box-row 0; ky=2 block box-row 63
                for bi in range(BH):
                    nc.sync.dma_start(
                        Bun1[0:27, 64 + bi * 4096: 64 + bi * 4096 + 64],
                        zt[0:27, 0:64])
                    nc.sync.dma_start(
                        Bun1[64:91, 64 + bi * 4096 + 63 * 64: 64 + bi * 4096 + 64 * 64],
                        zt[0:27, 0:64])

                bun1_v = Bun1[:, 64:64 + BH * 4096].rearrange(
                    "p (b r w) -> p b r w", b=BH, w=64)
                chunks = [(bi, hb) for bi in range(BH) for hb in range(8)]
                with tc.tile_pool(name="pp1", bufs=1, space="PSUM") as pp1:
                    for g in range(0, len(chunks), 8):
                        grp = chunks[g:g + 8]
                        pss = [pp1.tile([64, 512], F32, tag=f"ps1_{i}", name=f"ps1_{i}")
                               for i in range(len(grp))]
                        for ti, kxi in enumerate([1, 0, 2]):
                            for ci, (bi, hb) in enumerate(grp):
                                ps = pss[ci]
                                if kxi == 0:
                                    mv = bun1_v[:, bi, hb * 8:hb * 8 + 8, 0:63]
                                    ov = ps[:].rearrange("p (r w) -> p r w", w=64)[:, :, 1:64]
                                elif kxi == 1:
                                    mv = bun1_v[:, bi, hb * 8:hb * 8 + 8, :]
                                    ov = ps[:]
                                else:
                                    mv = bun1_v[:, bi, hb * 8:hb * 8 + 8, 1:64]
                                    ov = ps[:].rearrange("p (r w) -> p r w", w=64)[:, :, 0:63]
                                nc.tensor.matmul(ov, w1sb[:, kxi * 64:(kxi + 1) * 64], mv,
                                                 start=(ti == 0), stop=(ti == 2))
                        for ci, (bi, hb) in enumerate(grp):
                            b = imgs[bi]
                            maxpool_from_psum(nc, pss[ci][:], 4, 32,
                                              h1[:, b * 1024 + hb * 128: b * 1024 + (hb + 1) * 128])
        if (t := dbg_tap('h1', [64, 8192], F16)) is not None:
            nc.sync.dma_start(t.ap(), h1[:])

        # ============ L2 + L3 (pools coexist; halves cascade) ============
        with tc.tile_pool(name="l23w", bufs=1) as l23w, \
                tc.tile_pool(name="cons", bufs=1) as consp:
            w2sb = [l23w.tile([128 if i < 4 else 64, 1152], MMDT, tag=f"w2_{i}",
                              name=f"w2sb{i}") for i in range(5)]
            for i in range(5):
                nc.sync.dma_start(w2sb[i][:], w2_ext.ap()[i * 128:min(576, (i + 1) * 128), :])
            w3sb = [l23w.tile([128, 576], MMDT, tag=f"w3_{i}", name=f"w3sb{i}")
                    for i in range(9)]
            for i in range(9):
                nc.sync.dma_start(w3sb[i][:], w3_ext.ap()[i * 128:(i + 1) * 128, :])
            h1s = l23w.tile([128, 4096], F16)

            def do_l2_wave(wv):
                # wave = one image pair: (2bh, 4+2bh) for par=0, (2bh+1, 5+2bh)
                bh, par = wv // 2, wv % 2
                pair = [IMGS[bh][par], IMGS[bh][2 + par]]  # (top-row, bottom-row)
                col0 = bh * 2048 + par * 1024  # h1s col window [col0, col0+1024)
                if par == 0:
                    # copy the whole half's window once (both pairs)
                    nc.sync.dma_start(h1s[0:64, bh * 2048:(bh + 1) * 2048],
                                      h1[:, 2 * bh * 1024:(2 * bh + 2) * 1024])
                    nc.sync.dma_start(h1s[64:128, bh * 2048:(bh + 1) * 2048],
                                      h1[:, (4 + 2 * bh) * 1024:(6 + 2 * bh) * 1024])
                Ts2 = consp.tile([64, 2048], MMDT, tag=f"ts2_{par}", name=f"Ts2_{par}")
                for ii, b in enumerate(pair):
                    nc.scalar.activation(Ts2[:, ii * 1024:(ii + 1) * 1024],
                                         h1[:, b * 1024:(b + 1) * 1024], AF.Silu)
                T2 = [consp.tile([128, 2048], MMDT, tag=f"t2_{i}_{par}",
                                 name=f"T2_{i}_{par}") for i in range(4)]

                # h1s col window: [0:512) = top rows img pair[0] / bottom pair[1]
                for ck in range(2):
                    E = 512
                    D3t = emit_d3(nc, tpool, d3pool,
                                  h1s[:, col0 + ck * 512: col0 + (ck + 1) * 512],
                                  128, E, bias_tiles, name="l2")
                    D4 = d4pool.tile([128, NB * E], MMDT, tag="d4_l2", name="D4")
                    nc.gpsimd.tensor_tensor(D4[:], D3t[:, 0:NB * E],
                                            D3t[:, E:ND3 * E], op=OP.subtract)
                    for ph in range(2):
                        off = ph * 1024 + ck * E
                        for j in range(NB):
                            nc.sync.dma_start(
                                T2[j // 2][(j % 2) * 64:(j % 2) * 64 + 64,
                                           off:off + E],
                                D4[ph * 64:(ph + 1) * 64, j * E:(j + 1) * E])

                t2v = [T2[i][:].rearrange("p (b h w) -> p b h w", b=2, w=32)
                       for i in range(4)]
                ts2v = Ts2[:].rearrange("p (b h w) -> p b h w", b=2, w=32)
                taps = [(0, 1, 1)] + [(kt, kyi, kxi) for kt in range(5)
                                      for kyi in range(3) for kxi in range(3)
                                      if (kt, kyi, kxi) != (0, 1, 1)]
                n_taps = len(taps)
                chunks = [(ii, half) for ii in range(2) for half in range(2)]
                with tc.tile_pool(name="pp2", bufs=1, space="PSUM") as pp2:
                    pss = [pp2.tile([128, 512], F32, tag=f"ps2_{i}_{par}",
                                    name=f"ps2_{i}_{par}") for i in range(4)]
                    for tapi, (kt, kyi, kxi) in enumerate(taps):
                        for ci, (ii, half) in enumerate(chunks):
                            h0 = half * 16
                            ps = pss[ci]
                            r_lo = max(0, 1 - kyi - h0)
                            r_hi = min(16, 33 - h0 - kyi)
                            w_lo = 1 if kxi == 0 else 0
                            w_hi = 31 if kxi == 2 else 32
                            in_row = h0 + r_lo + kyi - 1
                            in_col = w_lo + kxi - 1
                            src = t2v[kt] if kt < 4 else ts2v
                            mv = src[:, ii, in_row:in_row + (r_hi - r_lo),
                                     in_col:in_col + (w_hi - w_lo)]
                            ov = ps[:].rearrange("p (r w) -> p r w", w=32)[
                                :, r_lo:r_hi, w_lo:w_hi]
                            nc.tensor.matmul(
                                ov,
                                w2sb[kt][:, (kyi * 3 + kxi) * 128:
                                         (kyi * 3 + kxi + 1) * 128],
                                mv, start=(tapi == 0),
                                stop=(tapi == n_taps - 1))
                    for ci, (ii, half) in enumerate(chunks):
                        b = pair[ii]
                        maxpool_from_psum(nc, pss[ci][:], 8, 16,
                                          h2[:, b * 256 + half * 128:
                                             b * 256 + (half + 1) * 128])

            def do_l3_half(bh):
                imgs = IMGS[bh]
                Ts3 = consp.tile([128, 1024], MMDT, tag=f"ts3_{bh}", name=f"Ts3_{bh}")
                for p in range(2):
                    b0 = imgs[2 * p]
                    nc.scalar.activation(Ts3[:, p * 512:(p + 1) * 512],
                                         h2[:, b0 * 256:(b0 + 2) * 256], AF.Silu)
                T3 = [consp.tile([128, 1024], MMDT, tag=f"t3_{j}", name=f"T3_{j}")
                      for j in range(NB)]

                for ck in range(2):
                    E = 512
                    b0 = imgs[2 * ck]
                    D3t = emit_d3(nc, tpool, d3pool,
                                  h2[:, b0 * 256:(b0 + 2) * 256],
                                  128, E, bias_tiles, name="l3")
                    for j in range(NB):
                        nc.gpsimd.tensor_tensor(
                            T3[j][:, ck * 512:(ck + 1) * 512],
                            D3t[:, j * E:(j + 1) * E],
                            D3t[:, (j + 1) * E:(j + 2) * E], op=OP.subtract)

                t3v = [T3[j][:].rearrange("p (b h w) -> p b h w", b=4, w=16)
                       for j in range(NB)]
                ts3v = Ts3[:].rearrange("p (b h w) -> p b h w", b=4, w=16)
                taps = [(0, 1, 1)] + [(kt, kyi, kxi) for kt in range(9)
                                      for kyi in range(3) for kxi in range(3)
                                      if (kt, kyi, kxi) != (0, 1, 1)]
                n_taps = len(taps)
                with tc.tile_pool(name="pp3", bufs=1, space="PSUM") as pp3:
                    pss = [pp3.tile([64, 512], F32, tag=f"ps3_{i}", name=f"ps3_{i}")
                           for i in range(2)]
                    for tapi, (kt, kyi, kxi) in enumerate(taps):
                        for ckc in range(2):
                            b0 = ckc * 2
                            ps = pss[ckc]
                            r_lo = max(0, 1 - kyi)
                            r_hi = min(16, 17 - kyi)
                            w_lo = 1 if kxi == 0 else 0
                            w_hi = 15 if kxi == 2 else 16
                            src = t3v[kt] if kt < 8 else ts3v
                            mv = src[:, b0:b0 + 2, r_lo + kyi - 1:r_hi + kyi - 1,
                                     w_lo + kxi - 1:w_lo + kxi - 1 + (w_hi - w_lo)]
                            ov = ps[:].rearrange("p (b r w) -> p b r w", b=2, w=16)[
                                :, :, r_lo:r_hi, w_lo:w_hi]
                            nc.tensor.matmul(
                                ov,
                                w3sb[kt][:, (kyi * 3 + kxi) * 64:
                                         (kyi * 3 + kxi + 1) * 64],
                                mv, start=(tapi == 0),
                                stop=(tapi == n_taps - 1))
                    for ckc in range(2):
                        b0 = imgs[2 * ckc]
                        maxpool_from_psum(nc, pss[ckc][:], 16, 8,
                                          h3[:, b0 * 64:(b0 + 2) * 64])

            do_l2_wave(0)
            do_l2_wave(1)
            do_l2_wave(2)
            do_l2_wave(3)
            do_l3_half(0)
            do_l3_half(1)
        if (t := dbg_tap('h2', [128, 2048])) is not None:
            nc.sync.dma_start(t.ap(), h2[:])
        if (t := dbg_tap('h3', [64, 512])) is not None:
            nc.sync.dma_start(t.ap(), h3[:])

        # ================= Linear =================
        with tc.tile_pool(name="linp", bufs=1) as linp:
            sl3 = linp.tile([64, 512], MMDT)
            nc.scalar.activation(sl3[:], h3[:], AF.Silu)
            D3L = emit_d3(nc, tpool, d3pool, h3[:], 64, 512, bias_tiles,
                          name="lin")
            D4L = linp.tile([64, NB * 512], MMDT)
            nc.vector.tensor_tensor(D4L[:], D3L[:, 0:NB * 512],
                                    D3L[:, 512:ND3 * 512], op=OP.subtract)
            if (t := dbg_tap('d4l', [64, NB * 512], MMDT)) is not None:
                nc.sync.dma_start(t.ap(), D4L[:])
            # pack j-pairs onto 128 partitions: rows 0:64 even j, 64:128 odd j
            D4P = linp.tile([128, 4 * 512], MMDT)
            d4l_v = D4L[:].rearrange("p (j b yx) -> p j b yx", j=NB, b=B)
            d4p_v = D4P[:].rearrange("p (pr b yx) -> p pr b yx", pr=4, b=B)
            nc.sync.dma_start(d4p_v[0:64, :, :, :], d4l_v[:, 0::2, :, :])
            nc.sync.dma_start(d4p_v[64:128, :, :, :], d4l_v[:, 1::2, :, :])

            with tc.tile_pool(name="wlpool", bufs=2) as wlp, \
                    tc.tile_pool(name="ppl", bufs=1, space="PSUM") as plin:
                psl = plin.tile([B, O_OUT], F32)
                sl3_v = sl3[:].rearrange("p (b yx) -> p b yx", b=B)
                first = True
                for piece in range(4):
                    wlt = wlp.tile([128, 6400], MMDT, tag="wl_piece", name="wlt")
                    nc.sync.dma_start(wlt[:], wlp_ext.ap()[:, piece * 6400:(piece + 1) * 6400])
                    wbt = wlp.tile([64, 1600], MMDT, tag="wlb_piece", name="wbt")
                    nc.sync.dma_start(wbt[:], wlb_ext.ap()[:, piece * 1600:(piece + 1) * 1600])
                    for yi in range(16):
                        yx = piece * 16 + yi
                        for pr in range(4):
                            nc.tensor.matmul(
                                psl[:], d4p_v[:, pr, :, yx],
                                wlt[:, (yi * 4 + pr) * O_OUT:(yi * 4 + pr + 1) * O_OUT],
                                start=first, stop=False)
                            first = False
                        nc.tensor.matmul(
                            psl[:], sl3_v[:, :, yx],
                            wbt[:, yi * O_OUT:(yi + 1) * O_OUT],
                            start=False, stop=(piece == 3 and yi == 15))
                osb = linp.tile([B, O_OUT], F32)
                nc.vector.tensor_copy(osb[:], psl[:])
                nc.sync.dma_start(out_ext.ap(), osb[:])

    nc.compile()
    return nc

# ===================================================================== runner
from concourse.bass_utils import run_bass_kernel_spmd

_NC_CACHE = {}


def _get_nc():
    if 'nc' not in _NC_CACHE:
        _NC_CACHE['nc'] = build_nc(dbg=())
    return _NC_CACHE['nc']


def kernel(x, wb1, ws1, wb2, ws2, wb3, ws3, lb, lc):
    """Full-input entry point: x [64,3,64,64] f32 -> out [64,100] f32.
    Shards the batch over 8 NeuronCores (8 samples each), replicating weights."""
    x = np.ascontiguousarray(np.asarray(x, dtype=np.float32))
    w = fold_weights(np.asarray(wb1, np.float32), np.asarray(ws1, np.float32),
                     np.asarray(wb2, np.float32), np.asarray(ws2, np.float32),
                     np.asarray(wb3, np.float32), np.asarray(ws3, np.float32),
                     np.asarray(lb, np.float32), np.asarray(lc, np.float32))
    nc = _get_nc()
    in_maps = [{'x': x[i * B:(i + 1) * B], **w} for i in range(8)]
    res = run_bass_kernel_spmd(nc, in_maps, core_ids=list(range(8)))
    return np.concatenate([res.results[i]['out'] for i in range(8)], axis=0)
